# revision 15
# baseline (speedup 1.0000x reference)
"""GPT layer (B=2, S=2048, D=768, H=12, DK=64, HID=3072, causal) on 8 TRN2 cores.

Sharding: cores 0-3 handle batch 0, cores 4-7 batch 1. Within a 4-core group:
tensor-parallel attention over heads (3 heads/core); the W_o partial product is
ReduceScattered per 512-row query chunk (4 small collectives, pipelined against
the next chunk's attention); each core then runs LN2 + full-width MLP on its
own 4x128 rows.

All matmuls run in bf16 (fp32 PSUM accumulation). LayerNorm1's mean
subtraction is folded into the QKV projections host-side via the centering
matrix (W_eff = C @ (g*W)), so the device only needs the per-row 1/std, which
is computed from the transposed activations x^T with ones-vector matmuls
(sum / sum-of-squares) and a table-free Newton rsqrt on the vector engine.
Softmax skips max-subtraction (scores are O(1) by construction); its
denominator comes from a ones-column appended to V; the causal mask is a
multiplicative 0/1 bf16 mask applied in-place to the single 128-wide diagonal
block of each diagonal score tile.
"""

import math
import os
from contextlib import ExitStack

import numpy as np
import ml_dtypes

import concourse.bass as bass
import concourse.tile as tile
from concourse import bacc, mybir
from concourse.bass_utils import run_bass_kernel_spmd
from concourse.masks import make_identity


F32 = mybir.dt.float32
BF16 = mybir.dt.bfloat16
I32 = mybir.dt.int32
AF = mybir.ActivationFunctionType
ALU = mybir.AluOpType
NPBF16 = ml_dtypes.bfloat16

B, S, D, H, DK, HID = 2, 2048, 768, 12, 64, 3072
EPS = 1e-5
G = 4            # cores per batch group
HG = H // G      # heads per core (3)
R = S // G       # rows per core (512)
NT = S // 128    # seq tiles (16)
DT = D // 128    # d tiles (6)
HT = HID // 128  # hid tiles (24)
CH = 4           # query chunks of 512
LEAD = 2         # score->AV software pipeline depth
MAGIC = 0x5F3759DF

_cache = {}


def _rsqrt_dve(nc, pool, out_ap, v_ap, magic_ap, n, tagp):
    """out = 1/sqrt(v) via int bit-trick + 2 Newton iterations, DVE only."""
    ti = pool.tile([128, n], I32, tag=f"{tagp}_i")
    nc.vector.tensor_scalar(
        ti[:], v_ap.bitcast(I32), 1, None, op0=ALU.logical_shift_right
    )
    y0 = pool.tile([128, n], I32, tag=f"{tagp}_y")
    nc.vector.tensor_sub(y0[:], magic_ap, ti[:])
    y = y0[:].bitcast(F32)
    for it in range(2):
        y2 = pool.tile([128, n], F32, tag=f"{tagp}_a")
        nc.vector.tensor_mul(y2[:], y, y)
        vy2 = pool.tile([128, n], F32, tag=f"{tagp}_b")
        nc.vector.tensor_mul(vy2[:], y2[:], v_ap)
        hh = pool.tile([128, n], F32, tag=f"{tagp}_c")
        nc.vector.tensor_scalar(
            hh[:], vy2[:], -0.5, 1.5, op0=ALU.mult, op1=ALU.add
        )
        yn = pool.tile([128, n], F32, tag=f"{tagp}_d")
        nc.vector.tensor_mul(yn[:], y, hh[:])
        y = yn[:]
    nc.vector.tensor_copy(out_ap, y)


def _build():
    if "nc" in _cache:
        return _cache["nc"]
    nc = bacc.Bacc("TRN2", target_bir_lowering=False, num_devices=8)

    xT_d = nc.dram_tensor("xT", [128, DT * S], BF16, kind="ExternalInput")
    xr_d = nc.dram_tensor("xr", [R, D], BF16, kind="ExternalInput")
    wqk_d = nc.dram_tensor("wqk", [128, DT * HG * 128], BF16, kind="ExternalInput")
    wv_d = nc.dram_tensor("wv", [128, DT * HG * 64], BF16, kind="ExternalInput")
    wo3_d = nc.dram_tensor("wo3", [64, HG * D], BF16, kind="ExternalInput")
    w1_d = nc.dram_tensor("w1", [128, HT * DT * 128], BF16, kind="ExternalInput")
    b1_d = nc.dram_tensor("b1r", [128, HT], F32, kind="ExternalInput")
    w2_d = nc.dram_tensor("w2", [HID, D], BF16, kind="ExternalInput")
    b2_d = nc.dram_tensor("b2r", [1, D], BF16, kind="ExternalInput")
    mask_d = nc.dram_tensor("mask", [128, 128], BF16, kind="ExternalInput")
    out_d = nc.dram_tensor("out", [R, D], F32, kind="ExternalOutput")

    with tile.TileContext(nc) as tc, ExitStack() as top:
        consts = top.enter_context(tc.tile_pool(name="consts", bufs=1))
        dram = top.enter_context(tc.tile_pool(name="dram", bufs=1, space="DRAM"))
        # outer pools that live across attention + MLP
        mpool = top.enter_context(tc.tile_pool(name="mpool", bufs=1))
        mstats = top.enter_context(tc.tile_pool(name="mstats", bufs=4))
        mscratch = top.enter_context(tc.tile_pool(name="mscratch", bufs=2))
        ps_t2 = top.enter_context(tc.tile_pool(name="ps_t2", bufs=1, space="PSUM"))

        ident_bf = consts.tile([128, 128], BF16)
        make_identity(nc, ident_bf[:])
        ones_bf1 = consts.tile([1, 128], BF16)
        nc.vector.memset(ones_bf1[:], 1.0)
        ones_col = consts.tile([128, 1], BF16)
        nc.vector.memset(ones_col[:], 1.0)
        magic_t = consts.tile([128, 4], I32)
        nc.vector.memset(magic_t[:], MAGIC)
        mask_sb = consts.tile([128, 128], BF16)
        nc.sync.dma_start(mask_sb[:], mask_d[:])
        wqk_sb = consts.tile([128, DT, HG, 128], BF16)
        nc.sync.dma_start(
            wqk_sb[:], wqk_d[:].rearrange("p (t h n) -> p t h n", t=DT, h=HG)
        )
        wv_sb = consts.tile([128, DT, HG, 64], BF16)
        nc.sync.dma_start(
            wv_sb[:], wv_d[:].rearrange("p (t h n) -> p t h n", t=DT, h=HG)
        )
        wo3_sb = consts.tile([64, HG, D], BF16)
        nc.sync.dma_start(wo3_sb[:], wo3_d[:].rearrange("p (h n) -> p h n", h=HG))
        b1_sb = consts.tile([128, HT], F32)
        nc.sync.dma_start(b1_sb[:], b1_d[:])
        b2bc = consts.tile([128, D], BF16)
        nc.sync.dma_start(
            b2bc[:],
            bass.AP(tensor=b2_d[:].tensor, offset=b2_d[:].offset, ap=[[0, 128], [1, D]]),
        )
        w2_sb = consts.tile([128, HT, D], BF16)

        party = [dram.tile([2 * R, D], BF16, name=f"party{c}") for c in range(2)]
        rsd = [dram.tile([256, D], BF16, name=f"rsd{c}") for c in range(2)]
        sq_d = [dram.tile([2, 512], F32, name=f"sq_d{c}") for c in range(CH)]
        rcol_d = [dram.tile([128, 4], BF16, name=f"rcol_d{c}") for c in range(CH)]

        y_sb = mpool.tile([128, 4, D], BF16)
        h2T = mpool.tile([128, DT, R], BF16)
        gT = mpool.tile([128, HT, R], BF16)

        def ln2_tile(m):
            rs_t = mscratch.tile([128, D], BF16, tag="rst")
            nc.sync.dma_start(
                rs_t[:], rsd[m // 2][(m % 2) * 128:(m % 2) * 128 + 128, :]
            )
            xr_t = mscratch.tile([128, D], BF16, tag="xrt")
            nc.sync.dma_start(xr_t[:], xr_d[m * 128:(m + 1) * 128, :])
            nc.vector.tensor_add(y_sb[:, m, :], rs_t[:], xr_t[:])
            bn6 = mstats.tile([128, 3, 6], F32, tag="bn6")
            for sg in range(3):
                nc.vector.bn_stats(
                    bn6[:, sg, :], y_sb[:, m, sg * 256:(sg + 1) * 256]
                )
            mv = mstats.tile([128, 2], F32, tag="mv")
            nc.vector.bn_aggr(mv[:], bn6[:])
            v2 = mstats.tile([128, 1], F32, tag="v2")
            nc.vector.tensor_scalar_add(v2[:], mv[:, 1:2], EPS)
            rstd2 = mstats.tile([128, 1], F32, tag="rstd2")
            _rsqrt_dve(nc, mstats, rstd2[:], v2[:], magic_t[:, 0:1], 1, "m2")
            h2_t = mscratch.tile([128, D], BF16, tag="h2row")
            nc.vector.tensor_scalar(
                h2_t[:], y_sb[:, m, :], mv[:, 0:1], rstd2[:],
                op0=ALU.subtract, op1=ALU.mult,
            )
            for half in range(2):
                pt = ps_t2.tile([128, 384], BF16, tag="ptr", bufs=1)
                for k in range(3):
                    dt = half * 3 + k
                    nc.tensor.transpose(
                        pt[:, k * 128:(k + 1) * 128],
                        h2_t[:, dt * 128:(dt + 1) * 128], ident_bf[:],
                    )
                nc.vector.tensor_copy(
                    h2T[:, half * 3:half * 3 + 3, m * 128:(m + 1) * 128], pt[:]
                )

        with ExitStack() as attn_scope:
            apool = attn_scope.enter_context(tc.tile_pool(name="apool", bufs=1))
            stats = attn_scope.enter_context(tc.tile_pool(name="stats", bufs=2))
            scratch = attn_scope.enter_context(tc.tile_pool(name="scratch", bufs=3))
            epool = attn_scope.enter_context(tc.tile_pool(name="epool", bufs=4))
            qpool = attn_scope.enter_context(tc.tile_pool(name="qpool", bufs=2))
            ps_sc = attn_scope.enter_context(
                tc.tile_pool(name="ps_sc", bufs=2, space="PSUM")
            )
            ps_o = attn_scope.enter_context(
                tc.tile_pool(name="ps_o", bufs=2, space="PSUM")
            )
            ps_qkv = attn_scope.enter_context(
                tc.tile_pool(name="ps_qkv", bufs=1, space="PSUM")
            )

            xT_sb = apool.tile([128, DT, S], BF16)
            for dt in range(DT):
                nc.sync.dma_start(
                    xT_sb[:, dt, 0:512], xT_d[:, dt * S: dt * S + 512]
                )
            for dt in range(DT):
                nc.sync.dma_start(
                    xT_sb[:, dt, 512:S], xT_d[:, dt * S + 512:(dt + 1) * S]
                )
            # W2 prefetch into SBUF on the GpSimd issue queue (before any
            # collective is enqueued there)
            for t in range(0, HT, 4):
                nc.gpsimd.dma_start(
                    w2_sb[:, t:t + 4, :],
                    w2_d[t * 128:(t + 4) * 128, :]
                    .rearrange("(t p) n -> p t n", p=128),
                )

            KT = apool.tile([64, HG, S], BF16)
            Vg = apool.tile([128, NT, HG, DK + 1], BF16)
            nc.vector.memset(Vg[:, :, :, DK:DK + 1], 1.0)
            rstd_bc = apool.tile([64, S], F32)
            rstd_all = apool.tile([128, NT], F32)
            rstd_allb = apool.tile([128, NT], BF16)
            rsT = apool.tile([1, S], BF16)

            # ---- LN1 statistics for one chunk, from x^T via ones-matmuls ----
            def ln1_stats_a(c):
                cs = c * 512
                srow = ps_sc.tile([1, 512], F32, tag="psc", bufs=2, name="srow")
                for dt in range(DT):
                    nc.tensor.matmul(
                        srow[:], ones_col[:], xT_sb[:, dt, cs:cs + 512],
                        start=(dt == 0), stop=(dt == DT - 1),
                    )
                s_sb = stats.tile([1, 512], F32, tag="srow")
                nc.vector.tensor_copy(s_sb[:], srow[:])
                nc.sync.dma_start(sq_d[c][0:1, :], s_sb[:])

            def ln1_stats_b(c):
                cs = c * 512
                qrow = ps_sc.tile([1, 512], F32, tag="psc", bufs=2, name="qrow")
                for dt in range(DT):
                    sq = scratch.tile([128, 512], BF16, tag="sq")
                    nc.vector.tensor_mul(
                        sq[:], xT_sb[:, dt, cs:cs + 512], xT_sb[:, dt, cs:cs + 512]
                    )
                    nc.tensor.matmul(
                        qrow[:], ones_col[:], sq[:],
                        start=(dt == 0), stop=(dt == DT - 1),
                    )
                q_sb = stats.tile([1, 512], F32, tag="qrow")
                nc.vector.tensor_copy(q_sb[:], qrow[:])
                nc.sync.dma_start(sq_d[c][1:2, :], q_sb[:])
                # rows -> column layout [128, 4] via a small DRAM bounce
                scol = stats.tile([128, 8], F32, tag="scol")
                nc.sync.dma_start(
                    scol[:, 0:4],
                    sq_d[c][0:1, :].rearrange("a (t q) -> (a q) t", t=4, q=128),
                )
                nc.sync.dma_start(
                    scol[:, 4:8],
                    sq_d[c][1:2, :].rearrange("a (t q) -> (a q) t", t=4, q=128),
                )
                cD = 1.0 / D
                a_t = stats.tile([128, 4], F32, tag="a")
                nc.vector.tensor_scalar_mul(a_t[:], scol[:, 0:4], cD)
                a2 = stats.tile([128, 4], F32, tag="a2")
                nc.vector.tensor_mul(a2[:], a_t[:], a_t[:])
                qe = stats.tile([128, 4], F32, tag="qe")
                nc.vector.tensor_scalar(
                    qe[:], scol[:, 4:8], cD, EPS, op0=ALU.mult, op1=ALU.add
                )
                v_t = stats.tile([128, 4], F32, tag="v")
                nc.vector.tensor_sub(v_t[:], qe[:], a2[:])
                _rsqrt_dve(
                    nc, stats, rstd_all[:, c * 4:(c + 1) * 4], v_t[:],
                    magic_t[:, 0:4], 4, "m1",
                )
                # column -> row layout via a small DRAM bounce
                nc.vector.tensor_copy(
                    rstd_allb[:, c * 4:(c + 1) * 4], rstd_all[:, c * 4:(c + 1) * 4]
                )
                nc.sync.dma_start(rcol_d[c][:], rstd_allb[:, c * 4:(c + 1) * 4])
                nc.sync.dma_start(
                    rsT[:, cs:cs + 512].rearrange("p (t q) -> p t q", t=4, q=128),
                    rcol_d[c][:].rearrange("q t -> t q"),
                )
                # broadcast to 64 partitions via a K=1 matmul (keeps GpSimd
                # free: collectives block its queue)
                bcp = ps_qkv.tile([64, 512], F32, tag="pqkv", bufs=1, name="bcp")
                nc.tensor.matmul(
                    bcp[:], ones_bf1[:, 0:64], rsT[:, cs:cs + 512],
                    start=True, stop=True,
                )
                nc.vector.tensor_copy(rstd_bc[:, cs:cs + 512], bcp[:])

            def emit_pqk(c, h, QT):
                cs = c * 512
                pqk = ps_qkv.tile([128, 512], F32, tag="pqkv", bufs=1, name="pqk")
                for dt in range(DT):
                    nc.tensor.matmul(
                        pqk[:], wqk_sb[:, dt, h, :], xT_sb[:, dt, cs:cs + 512],
                        start=(dt == 0), stop=(dt == DT - 1),
                    )
                nc.vector.tensor_mul(
                    QT[:, h, :], pqk[0:64, :], rstd_bc[:, cs:cs + 512]
                )
                nc.vector.tensor_mul(
                    KT[:, h, cs:cs + 512], pqk[64:128, :], rstd_bc[:, cs:cs + 512]
                )

            def emit_pv(c, j):
                st = 4 * c + j
                pv = ps_qkv.tile([128, HG * 64], F32, tag="pqkv", bufs=1, name="pv")
                for dt in range(DT):
                    nc.tensor.matmul(
                        pv[:, 0:HG * 64], xT_sb[:, dt, st * 128:(st + 1) * 128],
                        wv_sb[:, dt, :, :],
                        start=(dt == 0), stop=(dt == DT - 1),
                    )
                nc.vector.tensor_scalar(
                    Vg[:, st, :, 0:DK],
                    pv[:, 0:HG * 64].rearrange("p (h n) -> p h n", h=HG),
                    rstd_all[:, st:st + 1], None, op0=ALU.mult,
                )

            # chunk 0 front: stats + QKV emitted directly
            ln1_stats_a(0)
            ln1_stats_b(0)
            QT_cur = qpool.tile([64, HG, 512], BF16, tag="qt", name="QT0")
            for h in range(HG):
                emit_pqk(0, h, QT_cur)
            for j in range(4):
                emit_pv(0, j)

            for c in range(CH):
                cs = c * 512
                QT = QT_cur
                OT = qpool.tile([64, HG, 512], BF16, tag="ot", name=f"OT{c}")
                fillers = []
                if c + 1 < CH:
                    QT_next = qpool.tile(
                        [64, HG, 512], BF16, tag="qt", name=f"QT{c + 1}"
                    )
                    fillers.append(lambda c=c: ln1_stats_a(c + 1))
                    fillers.append(lambda c=c: ln1_stats_b(c + 1))
                    for h in range(HG):
                        fillers.append(
                            lambda c=c, h=h, q=QT_next: emit_pqk(c + 1, h, q)
                        )
                    for j in range(4):
                        fillers.append(lambda c=c, j=j: emit_pv(c + 1, j))
                else:
                    QT_next = None
                if c == 3:
                    ln2_tile(0)
                    ln2_tile(1)

                # scores -> exp -> (mask) -> A@V; exp batched per tile
                # pair to amortize the ~352-cycle ACT instruction overhead.
                # Next-chunk QKV/stats matmuls are interleaved between pairs
                # as PE filler so the scalar engine never starves.
                ntl = 4 * (c + 1)
                npair = ntl // 2
                for h in range(HG):
                    po = ps_o.tile([DK + 1, 512], F32, tag="po", bufs=2)
                    es = {}
                    q0s = {}
                    for ip in range(npair + 1):
                        if ip < npair:
                            t0, t1 = 2 * ip, 2 * ip + 1
                            dd0, dd1 = t0 - 4 * c, t1 - 4 * c
                            q00 = dd0 * 128 if dd0 > 0 else 0
                            q01 = dd1 * 128 if dd1 > 0 else 0
                            q0s[t0], q0s[t1] = q00, q01
                            psc = ps_sc.tile([128, 1024], F32, tag="psc", bufs=2)
                            nc.tensor.matmul(
                                psc[:, q00:512],
                                KT[:, h, t0 * 128:(t0 + 1) * 128],
                                QT[:, h, q00:512], start=True, stop=True,
                            )
                            nc.tensor.matmul(
                                psc[:, 512 + q01:1024],
                                KT[:, h, t1 * 128:(t1 + 1) * 128],
                                QT[:, h, q01:512], start=True, stop=True,
                                skip_group_check=True,
                            )
                            e_t = epool.tile([128, 1024], BF16, tag="e", bufs=3)
                            nc.scalar.activation(
                                e_t[:, q00:1024], psc[:, q00:1024], AF.Exp
                            )
                            if dd0 >= 0:
                                nc.vector.tensor_mul(
                                    e_t[:, q00:q00 + 128], e_t[:, q00:q00 + 128],
                                    mask_sb[:],
                                )
                            if dd1 >= 0:
                                nc.vector.tensor_mul(
                                    e_t[:, 512 + q01:512 + q01 + 128],
                                    e_t[:, 512 + q01:512 + q01 + 128],
                                    mask_sb[:],
                                )
                            es[ip] = e_t
                        jp = ip - 1
                        if jp >= 0:
                            e_p = es.pop(jp)
                            for tt, off in ((2 * jp, 0), (2 * jp + 1, 512)):
                                jq0 = q0s.pop(tt)
                                nc.tensor.matmul(
                                    po[:, jq0:512], Vg[:, tt, h, :],
                                    e_p[:, off + jq0:off + 512],
                                    start=(tt == 0), stop=(tt == ntl - 1),
                                )
                        if fillers:
                            fillers.pop(0)()
                    # normalize: OT = po[:DK] * broadcast(1/po[DK])
                    den_s = stats.tile([1, 512], F32, tag="den")
                    nc.vector.tensor_copy(den_s[:], po[DK:DK + 1, :])
                    rec_f = stats.tile([1, 512], F32, tag="rec_f")
                    nc.vector.reciprocal_approx_fast(rec_f[:], den_s[:])
                    rec = stats.tile([1, 512], BF16, tag="rec")
                    nc.vector.tensor_copy(rec[:], rec_f[:])
                    rbp = ps_o.tile([64, 512], F32, tag="po", bufs=2, name="rbp")
                    nc.tensor.matmul(
                        rbp[:], ones_bf1[:, 0:64], rec[:],
                        start=True, stop=True,
                    )
                    rb = epool.tile([64, 512], BF16, tag="rb", bufs=2)
                    nc.vector.tensor_copy(rb[:], rbp[:])
                    nc.vector.tensor_mul(OT[:, h, :], po[0:DK, :], rb[:])
                while fillers:
                    fillers.pop(0)()

                # W_o partial for this chunk's 4 row tiles
                for j in range(4):
                    row0 = (c % 2) * 512 + j * 128
                    prow = scratch.tile([128, D], BF16, tag="prow")
                    for n0, nw in ((0, 512), (512, 256)):
                        pw = ps_sc.tile([128, 512], F32, tag="psc", bufs=2, name="pw")
                        for hh in range(HG):
                            nc.tensor.matmul(
                                pw[:, 0:nw], OT[:, hh, j * 128:(j + 1) * 128],
                                wo3_sb[:, hh, n0:n0 + nw],
                                start=(hh == 0), stop=(hh == HG - 1),
                            )
                        nc.vector.tensor_copy(prow[:, n0:n0 + nw], pw[:, 0:nw])
                    nc.sync.dma_start(
                        party[c // 2][row0:row0 + 128, :], prow[:]
                    )

                if c % 2 == 1:
                    nc.gpsimd.collective_compute(
                        "ReduceScatter", ALU.add,
                        replica_groups=[[0, 1, 2, 3], [4, 5, 6, 7]],
                        ins=[party[c // 2][:].opt()], outs=[rsd[c // 2][:].opt()],
                    )
                QT_cur = QT_next

        # ---- MLP: fc1 in two row-halves (A: rows 0-255, B: 256-511) ----
        with ExitStack() as mlp_scope:
            w1pool = mlp_scope.enter_context(tc.tile_pool(name="w1pool", bufs=6))
            ps_f1 = mlp_scope.enter_context(
                tc.tile_pool(name="ps_f1", bufs=2, space="PSUM")
            )
            ps_f2 = mlp_scope.enter_context(
                tc.tile_pool(name="ps_f2", bufs=1, space="PSUM")
            )

            def fc1_half(hf):
                r0 = hf * 256
                for hc in range(HT):
                    w1c = w1pool.tile([128, DT, 128], BF16, tag="w1c")
                    nc.sync.dma_start(
                        w1c[:],
                        w1_d[:, hc * DT * 128:(hc + 1) * DT * 128]
                        .rearrange("p (t n) -> p t n", t=DT),
                    )
                    pf = ps_f1.tile([128, 256], F32, tag="pf", bufs=2)
                    for dt in range(DT):
                        nc.tensor.matmul(
                            pf[:], w1c[:, dt, :], h2T[:, dt, r0:r0 + 256],
                            start=(dt == 0), stop=(dt == DT - 1),
                        )
                    nc.scalar.activation(
                        gT[:, hc, r0:r0 + 256], pf[:], AF.Gelu,
                        bias=b1_sb[:, hc:hc + 1],
                    )

            def fc2_half(hf):
                pacc = {}
                for m in (2 * hf, 2 * hf + 1):
                    for n0, nw in ((0, 512), (512, 256)):
                        pacc[(m, n0)] = ps_f2.tile(
                            [128, nw], F32, tag=f"pf2_{m % 2}_{n0}", bufs=1,
                            name=f"pf2_{m}_{n0}",
                        )
                for t in range(HT):
                    for m in (2 * hf, 2 * hf + 1):
                        rl = m * 128
                        for n0, nw in ((0, 512), (512, 256)):
                            nc.tensor.matmul(
                                pacc[(m, n0)], gT[:, t, rl:rl + 128],
                                w2_sb[:, t, n0:n0 + nw],
                                start=(t == 0), stop=(t == HT - 1),
                            )
                for m in (2 * hf, 2 * hf + 1):
                    yb = mscratch.tile([128, D], BF16, tag="yb")
                    nc.vector.tensor_add(yb[:], y_sb[:, m, :], b2bc[:])
                    o_t = mscratch.tile([128, D], F32, tag="ot")
                    for n0, nw in ((0, 512), (512, 256)):
                        nc.vector.tensor_add(
                            o_t[:, n0:n0 + nw], pacc[(m, n0)], yb[:, n0:n0 + nw]
                        )
                    nc.sync.dma_start(
                        out_d[m * 128:(m + 1) * 128, :], o_t[:]
                    )

            fc1_half(0)
            ln2_tile(2)
            ln2_tile(3)
            fc2_half(0)
            fc1_half(1)
            fc2_half(1)

    nc.finalize()
    _cache["nc"] = nc
    return nc


def _mask_np():
    # mask[p, q] = 1 where key p <= query q within a 128x128 diagonal block
    p = np.arange(128)[:, None]
    q = np.arange(128)[None, :]
    return (p <= q).astype(NPBF16)


def kernel(x, Wq, Wk, Wv, Wo, W1, b1, W2, b2, g_ln1, b_ln1, g_ln2, b_ln2):
    x = np.asarray(x, dtype=np.float32)
    Wq = np.asarray(Wq, dtype=np.float32)
    Wk = np.asarray(Wk, dtype=np.float32)
    Wv = np.asarray(Wv, dtype=np.float32)
    Wo = np.asarray(Wo, dtype=np.float32)
    W1 = np.asarray(W1, dtype=np.float32)
    b1 = np.asarray(b1, dtype=np.float32)
    W2 = np.asarray(W2, dtype=np.float32)
    b2 = np.asarray(b2, dtype=np.float32)
    g_ln1 = np.asarray(g_ln1, dtype=np.float32)
    b_ln1 = np.asarray(b_ln1, dtype=np.float32)
    g_ln2 = np.asarray(g_ln2, dtype=np.float32)
    b_ln2 = np.asarray(b_ln2, dtype=np.float32)
    assert not np.any(b_ln1), "nonzero b_ln1 not supported by this kernel"

    nc = _build()
    mask = _mask_np()
    scale = 1.0 / math.sqrt(DK)

    # LN2 gain folds into W1 (exactly); LN2 bias folds into the fc1 bias.
    W1_eff = g_ln2[:, None] * W1
    b1_eff = b1 + b_ln2 @ W1
    w1_r = np.ascontiguousarray(
        W1_eff.reshape(DT, 128, HT, 128).transpose(1, 2, 0, 3).reshape(128, -1)
    ).astype(NPBF16)
    b1r = np.ascontiguousarray(b1_eff.reshape(HT, 128).T).astype(np.float32)
    w2_bf = W2.astype(NPBF16)
    b2r = b2.reshape(1, D).astype(NPBF16)

    in_maps = []
    for core in range(8):
        b, r = core // G, core % G
        hsl = slice(HG * r, HG * (r + 1))
        # [D, HG, 64] with LN1 gain folded in; Q side also folds 1/sqrt(dk).
        # LN1 mean-centering folds in as well: W_eff = C @ W  (C = I - 11^T/D)
        wq3 = (Wq[hsl] * g_ln1[None, :, None]).transpose(1, 0, 2) * scale
        wk3 = (Wk[hsl] * g_ln1[None, :, None]).transpose(1, 0, 2)
        wv3 = (Wv[hsl] * g_ln1[None, :, None]).transpose(1, 0, 2)
        wq3 = wq3 - wq3.mean(axis=0, keepdims=True)
        wk3 = wk3 - wk3.mean(axis=0, keepdims=True)
        wv3 = wv3 - wv3.mean(axis=0, keepdims=True)
        wqk = np.concatenate([wq3, wk3], axis=2)  # [D, HG, 128]
        wqk_r = np.ascontiguousarray(
            wqk.reshape(DT, 128, HG, 128).transpose(1, 0, 2, 3).reshape(128, -1)
        ).astype(NPBF16)
        wv_r = np.ascontiguousarray(
            wv3.reshape(DT, 128, HG, 64).transpose(1, 0, 2, 3).reshape(128, -1)
        ).astype(NPBF16)
        wo_c = Wo[HG * DK * r:HG * DK * (r + 1), :]
        wo3 = np.ascontiguousarray(
            wo_c.reshape(HG, DK, D).transpose(1, 0, 2).reshape(DK, HG * D)
        ).astype(NPBF16)
        xb = x[b].astype(NPBF16)
        xT_r = np.ascontiguousarray(
            xb.T.reshape(DT, 128, S).transpose(1, 0, 2).reshape(128, -1)
        )
        # core's MLP rows: 256 from each of the two ReduceScatter halves
        rows1 = slice(r * 256, (r + 1) * 256)
        rows2 = slice(1024 + r * 256, 1024 + (r + 1) * 256)
        xr = np.concatenate([xb[rows1], xb[rows2]], axis=0)
        in_maps.append({
            "xT": xT_r,
            "xr": np.ascontiguousarray(xr),
            "wqk": wqk_r, "wv": wv_r,
            "wo3": wo3,
            "w1": w1_r, "b1r": b1r, "w2": w2_bf, "b2r": b2r,
            "mask": mask,
        })

    trace = bool(int(os.environ.get("BENCH_TRACE", "0")))
    res = run_bass_kernel_spmd(nc, in_maps, core_ids=list(range(8)), trace=trace)
    _cache["last_results"] = res

    out = np.empty((B, S, D), dtype=np.float32)
    for core in range(8):
        b, r = core // G, core % G
        o = res.results[core]["out"]
        out[b, r * 256:(r + 1) * 256, :] = o[0:256]
        out[b, 1024 + r * 256:1024 + (r + 1) * 256, :] = o[256:512]
    return out


# revision 17
# speedup vs baseline: 1.0022x; 1.0022x over previous
"""GPT layer (B=2, S=2048, D=768, H=12, DK=64, HID=3072, causal) on 8 TRN2 cores.

Sharding: cores 0-3 handle batch 0, cores 4-7 batch 1. Within a 4-core group:
tensor-parallel attention over heads (3 heads/core); the W_o partial product is
ReduceScattered in two row-chunks (pipelined against attention / MLP); each
core then runs LN2 + full-width MLP on its own 512 rows (2x 256-row pieces).

All matmuls run in bf16 (fp32 PSUM accumulation). LayerNorm1's mean
subtraction is folded into the QKV projections via an extended contraction
(Q = rstd * (x@Wq - mu*colsum(Wq))), so the transposed activations x^T are
uploaded pre-transposed from the host and never normalized on-device.
Softmax skips max-subtraction (scores are O(1) by construction); its
denominator comes from a ones-column appended to V; the causal mask is a
multiplicative 0/1 bf16 mask applied in-place to the single 128-wide diagonal
block of each diagonal score tile. Score tiles are exp'd in PAIRS
([128,1024] PSUM) to amortize the ~352-cycle ACT instruction overhead.
"""

import math
import os
from contextlib import ExitStack

import numpy as np
import ml_dtypes

import concourse.bass as bass
import concourse.tile as tile
from concourse import bacc, mybir
from concourse.bass_utils import run_bass_kernel_spmd
from concourse.masks import make_identity


F32 = mybir.dt.float32
BF16 = mybir.dt.bfloat16
AF = mybir.ActivationFunctionType
ALU = mybir.AluOpType
NPBF16 = ml_dtypes.bfloat16

B, S, D, H, DK, HID = 2, 2048, 768, 12, 64, 3072
EPS = 1e-5
G = 4            # cores per batch group
HG = H // G      # heads per core (3)
R = S // G       # rows per core (512)
NT = S // 128    # seq tiles (16)
DT = D // 128    # d tiles (6)
HT = HID // 128  # hid tiles (24)
CH = 4           # query chunks of 512

_cache = {}


def _build():
    if "nc" in _cache:
        return _cache["nc"]
    nc = bacc.Bacc("TRN2", target_bir_lowering=False, num_devices=8)

    xrows_d = nc.dram_tensor("xrows", [S, D], BF16, kind="ExternalInput")
    xr_d = nc.dram_tensor("xr", [R, D], BF16, kind="ExternalInput")
    xT_d = nc.dram_tensor("xT", [128, DT * S], BF16, kind="ExternalInput")
    wqk_d = nc.dram_tensor("wqk", [128, DT * HG * 128], BF16, kind="ExternalInput")
    wv_d = nc.dram_tensor("wv", [128, DT * HG * 64], BF16, kind="ExternalInput")
    csqk_d = nc.dram_tensor("csqk", [1, HG * 128], BF16, kind="ExternalInput")
    csv_d = nc.dram_tensor("csv", [1, HG * 64], BF16, kind="ExternalInput")
    wo3_d = nc.dram_tensor("wo3", [64, HG * D], BF16, kind="ExternalInput")
    w1_d = nc.dram_tensor("w1", [128, HT * DT * 128], BF16, kind="ExternalInput")
    b1_d = nc.dram_tensor("b1r", [128, HT], F32, kind="ExternalInput")
    w2_d = nc.dram_tensor("w2", [HID, D], BF16, kind="ExternalInput")
    b2_d = nc.dram_tensor("b2r", [1, D], BF16, kind="ExternalInput")
    mask_d = nc.dram_tensor("mask", [128, 4 * 512], BF16, kind="ExternalInput")
    out_d = nc.dram_tensor("out", [R, D], F32, kind="ExternalOutput")

    with tile.TileContext(nc) as tc, ExitStack() as top:
        consts = top.enter_context(tc.tile_pool(name="consts", bufs=1))
        dram = top.enter_context(tc.tile_pool(name="dram", bufs=1, space="DRAM"))

        ident = consts.tile([128, 128], F32)
        make_identity(nc, ident[:])
        ident_bf = consts.tile([128, 128], BF16)
        make_identity(nc, ident_bf[:])
        ones_bf = consts.tile([1, 128], BF16)
        nc.vector.memset(ones_bf[:], 1.0)
        eps_sb = consts.tile([128, 1], F32)
        nc.vector.memset(eps_sb[:], EPS)
        mask_sb = consts.tile([128, 4, 512], BF16)
        nc.sync.dma_start(mask_sb[:], mask_d[:].rearrange("p (d q) -> p d q", q=512))
        wqk_sb = consts.tile([128, DT, HG, 128], BF16)
        nc.sync.dma_start(
            wqk_sb[:], wqk_d[:].rearrange("p (t h n) -> p t h n", t=DT, h=HG)
        )
        wv_sb = consts.tile([128, DT, HG, 64], BF16)
        nc.sync.dma_start(
            wv_sb[:], wv_d[:].rearrange("p (t h n) -> p t h n", t=DT, h=HG)
        )
        csqk_sb = consts.tile([1, HG, 128], BF16)
        nc.sync.dma_start(csqk_sb[:], csqk_d[:].rearrange("p (h n) -> p h n", h=HG))
        csv_sb = consts.tile([1, HG, 64], BF16)
        nc.sync.dma_start(csv_sb[:], csv_d[:].rearrange("p (h n) -> p h n", h=HG))
        wo3_sb = consts.tile([64, HG, D], BF16)
        nc.sync.dma_start(wo3_sb[:], wo3_d[:].rearrange("p (h n) -> p h n", h=HG))
        b1_sb = consts.tile([128, HT], F32)
        nc.sync.dma_start(b1_sb[:], b1_d[:])
        b2bc = consts.tile([128, D], BF16)
        nc.sync.dma_start(
            b2bc[:],
            bass.AP(tensor=b2_d[:].tensor, offset=b2_d[:].offset, ap=[[0, 128], [1, D]]),
        )

        party1 = dram.tile([2 * R, D], BF16)
        party2 = dram.tile([2 * R, D], BF16)
        rs1 = dram.tile([R // 2, D], BF16)
        rs2 = dram.tile([R // 2, D], BF16)

        with ExitStack() as attn_scope:
            apool = attn_scope.enter_context(tc.tile_pool(name="apool", bufs=1))
            stats = attn_scope.enter_context(tc.tile_pool(name="stats", bufs=8))
            scratch = attn_scope.enter_context(tc.tile_pool(name="scratch", bufs=3))
            epool = attn_scope.enter_context(tc.tile_pool(name="epool", bufs=4))
            ps_sc = attn_scope.enter_context(
                tc.tile_pool(name="ps_sc", bufs=1, space="PSUM")
            )
            ps_o = attn_scope.enter_context(
                tc.tile_pool(name="ps_o", bufs=1, space="PSUM")
            )
            ps_qkv = attn_scope.enter_context(
                tc.tile_pool(name="ps_qkv", bufs=1, space="PSUM")
            )
            ps_w = attn_scope.enter_context(
                tc.tile_pool(name="ps_w", bufs=1, space="PSUM")
            )

            xT_sb = apool.tile([128, DT, S], BF16)
            for dt in range(DT):
                nc.sync.dma_start(
                    xT_sb[:, dt, 0:512], xT_d[:, dt * S: dt * S + 512]
                )

            QT = apool.tile([64, HG, S], BF16)
            KT = apool.tile([64, HG, S], BF16)
            Vg = apool.tile([128, NT, HG, DK + 1], BF16)
            nc.vector.memset(Vg[:, :, :, DK:DK + 1], 1.0)
            OT = apool.tile([64, HG, S], BF16)
            rstd_bc = apool.tile([128, S], F32)
            muT = apool.tile([1, S], BF16)   # -mean, transposed to a row
            rsT = apool.tile([1, S], BF16)   # rstd, transposed to a row
            negmean_all = apool.tile([128, NT], F32)
            rstd_all = apool.tile([128, NT], F32)

            # ---- LN1 statistics from row-layout x (all upfront) ----
            for st in range(NT):
                xt = scratch.tile([128, D], BF16, tag="xin")
                nc.sync.dma_start(xt[:], xrows_d[st * 128:(st + 1) * 128, :])
                bn6 = stats.tile([128, 3, 6], F32, tag="bn6")
                for sg in range(3):
                    nc.vector.bn_stats(bn6[:, sg, :], xt[:, sg * 256:(sg + 1) * 256])
                mv = stats.tile([128, 2], F32, tag="mv")
                nc.vector.bn_aggr(mv[:], bn6[:])
                nc.vector.tensor_scalar_mul(
                    negmean_all[:, st:st + 1], mv[:, 0:1], -1.0
                )
                std = stats.tile([128, 1], F32, tag="std")
                nc.scalar.activation(std[:], mv[:, 1:2], AF.Sqrt, bias=eps_sb[:])
                nc.vector.reciprocal(rstd_all[:, st:st + 1], std[:])

            # remaining x^T chunks (chunk 0 was queued before the stats DMAs)
            for c in range(1, CH):
                for dt in range(DT):
                    nc.sync.dma_start(
                        xT_sb[:, dt, c * 512:(c + 1) * 512],
                        xT_d[:, dt * S + c * 512: dt * S + (c + 1) * 512],
                    )

            # transpose stats to single-partition rows (legal matmul operands),
            # then broadcast rstd along partitions via K=1 matmuls
            for c in range(CH):
                ptm = ps_w.tile([128, 512], F32, tag="pw", bufs=1)
                for j in range(4):
                    nc.tensor.transpose(
                        ptm[0:1, j * 128:(j + 1) * 128],
                        negmean_all[:, 4 * c + j:4 * c + j + 1], ident[:],
                    )
                nc.vector.tensor_copy(muT[:, c * 512:(c + 1) * 512], ptm[0:1, :])
                ptr = ps_w.tile([128, 512], F32, tag="pw", bufs=1)
                for j in range(4):
                    nc.tensor.transpose(
                        ptr[0:1, j * 128:(j + 1) * 128],
                        rstd_all[:, 4 * c + j:4 * c + j + 1], ident[:],
                    )
                nc.vector.tensor_copy(rsT[:, c * 512:(c + 1) * 512], ptr[0:1, :])
                pbx = ps_w.tile([128, 512], F32, tag="pw", bufs=1)
                for j in range(4):
                    nc.tensor.matmul(
                        pbx[:, j * 128:(j + 1) * 128], ones_bf[:],
                        rsT[:, (4 * c + j) * 128:(4 * c + j + 1) * 128],
                        start=True, stop=True, skip_group_check=True,
                    )
                nc.vector.tensor_copy(rstd_bc[:, c * 512:(c + 1) * 512], pbx[:])

            # ---- attention, chunk-pipelined over query blocks of 512 ----
            for c in range(CH):
                cs = c * 512
                # QK projections for this chunk (packed Q|K per head)
                for h in range(HG):
                    pqk = ps_qkv.tile([128, 512], F32, tag="pqk", bufs=1)
                    for dt in range(DT):
                        nc.tensor.matmul(
                            pqk[:], wqk_sb[:, dt, h, :], xT_sb[:, dt, cs:cs + 512],
                            start=(dt == 0), stop=False,
                        )
                    for j in range(4):
                        nc.tensor.matmul(
                            pqk[:, j * 128:(j + 1) * 128], csqk_sb[:, h, :],
                            muT[:, (4 * c + j) * 128:(4 * c + j + 1) * 128],
                            start=False, stop=True, skip_group_check=True,
                        )
                    nc.vector.tensor_mul(
                        QT[:, h, cs:cs + 512], pqk[0:64, :],
                        rstd_bc[0:64, cs:cs + 512],
                    )
                    nc.vector.tensor_mul(
                        KT[:, h, cs:cs + 512], pqk[64:128, :],
                        rstd_bc[64:128, cs:cs + 512],
                    )
                # V for the 4 key tiles of this chunk
                for j in range(4):
                    st = 4 * c + j
                    pv = ps_qkv.tile([128, HG * 64], F32, tag="pqk", bufs=1, name="pv")
                    for dt in range(DT):
                        nc.tensor.matmul(
                            pv[:], xT_sb[:, dt, st * 128:(st + 1) * 128],
                            wv_sb[:, dt, :, :],
                            start=(dt == 0), stop=False,
                        )
                    nc.tensor.matmul(
                        pv[:], muT[:, st * 128:(st + 1) * 128], csv_sb[:, :, :],
                        start=False, stop=True,
                    )
                    nc.vector.tensor_scalar(
                        Vg[:, st, :, 0:DK],
                        pv[:].rearrange("p (h n) -> p h n", h=HG),
                        rstd_all[:, st:st + 1], None, op0=ALU.mult,
                    )

                # scores -> exp -> (mask) -> A@V; exp batched per tile pair
                # to amortize the ~352-cycle ACT instruction overhead
                ntl = 4 * (c + 1)
                npair = ntl // 2
                for h in range(HG):
                    po = ps_o.tile([DK + 1, 512], F32, tag="po", bufs=2)
                    es = {}
                    q0s = {}
                    for ip in range(npair + 1):
                        if ip < npair:
                            t0, t1 = 2 * ip, 2 * ip + 1
                            dd0, dd1 = t0 - 4 * c, t1 - 4 * c
                            q00 = dd0 * 128 if dd0 > 0 else 0
                            q01 = dd1 * 128 if dd1 > 0 else 0
                            q0s[t0], q0s[t1] = q00, q01
                            psc = ps_sc.tile([128, 1024], F32, tag="psc", bufs=2)
                            nc.tensor.matmul(
                                psc[:, q00:512],
                                KT[:, h, t0 * 128:(t0 + 1) * 128],
                                QT[:, h, cs + q00:cs + 512],
                                start=True, stop=True,
                            )
                            nc.tensor.matmul(
                                psc[:, 512 + q01:1024],
                                KT[:, h, t1 * 128:(t1 + 1) * 128],
                                QT[:, h, cs + q01:cs + 512],
                                start=True, stop=True, skip_group_check=True,
                            )
                            e_t = epool.tile([128, 1024], BF16, tag="e", bufs=3)
                            nc.scalar.activation(
                                e_t[:, q00:1024], psc[:, q00:1024], AF.Exp
                            )
                            if dd0 >= 0:
                                nc.vector.tensor_mul(
                                    e_t[:, q00:q00 + 128], e_t[:, q00:q00 + 128],
                                    mask_sb[:, dd0, q00:q00 + 128],
                                )
                            if dd1 >= 0:
                                nc.vector.tensor_mul(
                                    e_t[:, 512 + q01:512 + q01 + 128],
                                    e_t[:, 512 + q01:512 + q01 + 128],
                                    mask_sb[:, dd1, q01:q01 + 128],
                                )
                            es[ip] = e_t
                        jp = ip - 1
                        if jp >= 0:
                            e_p = es.pop(jp)
                            for tt, off in ((2 * jp, 0), (2 * jp + 1, 512)):
                                jq0 = q0s.pop(tt)
                                nc.tensor.matmul(
                                    po[:, jq0:512], Vg[:, tt, h, :],
                                    e_p[:, off + jq0:off + 512],
                                    start=(tt == 0), stop=(tt == ntl - 1),
                                )
                    # normalize: OT = po[:DK] * broadcast(1/po[DK])
                    den_s = stats.tile([1, 512], F32, tag="den")
                    nc.vector.tensor_copy(den_s[:], po[DK:DK + 1, :])
                    rec_f = stats.tile([1, 512], F32, tag="rec_f")
                    nc.vector.reciprocal_approx_fast(rec_f[:], den_s[:])
                    rec = stats.tile([1, 512], BF16, tag="rec")
                    nc.vector.tensor_copy(rec[:], rec_f[:])
                    rb = epool.tile([64, 512], BF16, tag="rb", bufs=2)
                    nc.gpsimd.partition_broadcast(rb[:], rec[:])
                    nc.vector.tensor_mul(OT[:, h, cs:cs + 512], po[0:DK, :], rb[:])

                # W_o partial for this chunk's 4 row tiles
                for rt in range(4 * c, 4 * c + 4):
                    party_d = party1 if rt < 8 else party2
                    row0 = (rt % 8) * 128
                    for n0, nw in ((0, 512), (512, 256)):
                        pw = ps_w.tile([128, 512], F32, tag="pw", bufs=1)
                        for hh in range(HG):
                            nc.tensor.matmul(
                                pw[:, 0:nw], OT[:, hh, rt * 128:(rt + 1) * 128],
                                wo3_sb[:, hh, n0:n0 + nw],
                                start=(hh == 0), stop=(hh == HG - 1),
                            )
                        prow = scratch.tile([128, 512], BF16, tag="prow")
                        nc.vector.tensor_copy(prow[:, 0:nw], pw[:, 0:nw])
                        nc.sync.dma_start(
                            party_d[row0:row0 + 128, n0:n0 + nw], prow[:, 0:nw]
                        )

                if c == 1:
                    nc.gpsimd.collective_compute(
                        "ReduceScatter", ALU.add,
                        replica_groups=[[0, 1, 2, 3], [4, 5, 6, 7]],
                        ins=[party1[:].opt()], outs=[rs1[:].opt()],
                    )

        nc.gpsimd.collective_compute(
            "ReduceScatter", ALU.add,
            replica_groups=[[0, 1, 2, 3], [4, 5, 6, 7]],
            ins=[party2[:].opt()], outs=[rs2[:].opt()],
        )

        # ---- LN2 + MLP over two 256-row halves ----
        with ExitStack() as mlp_scope:
            mpool = mlp_scope.enter_context(tc.tile_pool(name="mpool", bufs=1))
            mstats = mlp_scope.enter_context(tc.tile_pool(name="mstats", bufs=8))
            mscratch = mlp_scope.enter_context(tc.tile_pool(name="mscratch", bufs=3))
            w1pool = mlp_scope.enter_context(tc.tile_pool(name="w1pool", bufs=3))
            w2pool = mlp_scope.enter_context(tc.tile_pool(name="w2pool", bufs=3))
            ps_t2 = mlp_scope.enter_context(
                tc.tile_pool(name="ps_t2", bufs=1, space="PSUM")
            )
            ps_f1 = mlp_scope.enter_context(
                tc.tile_pool(name="ps_f1", bufs=1, space="PSUM")
            )
            ps_f2 = mlp_scope.enter_context(
                tc.tile_pool(name="ps_f2", bufs=1, space="PSUM")
            )

            y_sb = mpool.tile([128, 4, D], BF16)
            h2T = mpool.tile([128, DT, R], BF16)
            gT = mpool.tile([128, HT, R], BF16)

            for hf in range(2):
                rs_d = rs1 if hf == 0 else rs2
                for m in range(2):
                    rl = hf * 256 + m * 128  # local row offset
                    rs_t = mscratch.tile([128, D], BF16, tag="rst")
                    nc.sync.dma_start(rs_t[:], rs_d[m * 128:(m + 1) * 128, :])
                    xr_t = mscratch.tile([128, D], BF16, tag="xrt")
                    nc.sync.dma_start(xr_t[:], xr_d[rl:rl + 128, :])
                    nc.vector.tensor_add(y_sb[:, hf * 2 + m, :], rs_t[:], xr_t[:])
                    bn6 = mstats.tile([128, 3, 6], F32, tag="bn6")
                    for sg in range(3):
                        nc.vector.bn_stats(
                            bn6[:, sg, :],
                            y_sb[:, hf * 2 + m, sg * 256:(sg + 1) * 256],
                        )
                    mv = mstats.tile([128, 2], F32, tag="mv")
                    nc.vector.bn_aggr(mv[:], bn6[:])
                    std = mstats.tile([128, 1], F32, tag="std")
                    nc.scalar.activation(std[:], mv[:, 1:2], AF.Sqrt, bias=eps_sb[:])
                    rstd = mstats.tile([128, 1], F32, tag="rstd")
                    nc.vector.reciprocal(rstd[:], std[:])
                    h2_t = mscratch.tile([128, D], BF16, tag="h2row")
                    nc.vector.tensor_scalar(
                        h2_t[:], y_sb[:, hf * 2 + m, :], mv[:, 0:1], rstd[:],
                        op0=ALU.subtract, op1=ALU.mult,
                    )
                    for half in range(2):
                        pt = ps_t2.tile([128, 384], BF16, tag="ptr", bufs=2)
                        for k in range(3):
                            dt = half * 3 + k
                            nc.tensor.transpose(
                                pt[:, k * 128:(k + 1) * 128],
                                h2_t[:, dt * 128:(dt + 1) * 128], ident_bf[:],
                            )
                        nc.vector.tensor_copy(
                            h2T[:, half * 3:half * 3 + 3, rl:rl + 128], pt[:]
                        )

                # fc1 + gelu for this half
                for hc in range(HT):
                    w1c = w1pool.tile([128, DT, 128], BF16, tag="w1c")
                    nc.sync.dma_start(
                        w1c[:],
                        w1_d[:, hc * DT * 128:(hc + 1) * DT * 128]
                        .rearrange("p (t n) -> p t n", t=DT),
                    )
                    pf = ps_f1.tile([128, 256], F32, tag="pf", bufs=2)
                    for dt in range(DT):
                        nc.tensor.matmul(
                            pf[:], w1c[:, dt, :],
                            h2T[:, dt, hf * 256:(hf + 1) * 256],
                            start=(dt == 0), stop=(dt == DT - 1),
                        )
                    nc.scalar.activation(
                        gT[:, hc, hf * 256:(hf + 1) * 256], pf[:], AF.Gelu,
                        bias=b1_sb[:, hc:hc + 1],
                    )

                # fc2 for this half (W2 streamed)
                pacc = {}
                for m in range(2):
                    for n0, nw in ((0, 512), (512, 256)):
                        pacc[(m, n0)] = ps_f2.tile(
                            [128, nw], F32, tag=f"pf2_{m}_{n0}", bufs=1,
                            name=f"pf2_{m}_{n0}",
                        )
                for t in range(HT):
                    w2t = w2pool.tile([128, D], BF16, tag="w2t")
                    nc.sync.dma_start(w2t[:], w2_d[t * 128:(t + 1) * 128, :])
                    for m in range(2):
                        rl = hf * 256 + m * 128
                        for n0, nw in ((0, 512), (512, 256)):
                            nc.tensor.matmul(
                                pacc[(m, n0)], gT[:, t, rl:rl + 128],
                                w2t[:, n0:n0 + nw],
                                start=(t == 0), stop=(t == HT - 1),
                            )
                for m in range(2):
                    yb = mscratch.tile([128, D], BF16, tag="yb")
                    nc.vector.tensor_add(yb[:], y_sb[:, hf * 2 + m, :], b2bc[:])
                    o_t = mscratch.tile([128, D], F32, tag="ot")
                    for n0, nw in ((0, 512), (512, 256)):
                        nc.vector.tensor_add(
                            o_t[:, n0:n0 + nw], pacc[(m, n0)], yb[:, n0:n0 + nw]
                        )
                    nc.sync.dma_start(
                        out_d[(hf * 2 + m) * 128:(hf * 2 + m + 1) * 128, :], o_t[:]
                    )

    nc.finalize()
    _cache["nc"] = nc
    return nc


def _mask_np():
    # mask[p, dd*512 + q] = 1 where key (dd*128 + p) <= query q, else 0
    m = np.zeros((128, 4 * 512), dtype=NPBF16)
    p = np.arange(128)[:, None]
    q = np.arange(512)[None, :]
    for dd in range(4):
        m[:, dd * 512:(dd + 1) * 512] = (dd * 128 + p <= q).astype(NPBF16)
    return m


def kernel(x, Wq, Wk, Wv, Wo, W1, b1, W2, b2, g_ln1, b_ln1, g_ln2, b_ln2):
    x = np.asarray(x, dtype=np.float32)
    Wq = np.asarray(Wq, dtype=np.float32)
    Wk = np.asarray(Wk, dtype=np.float32)
    Wv = np.asarray(Wv, dtype=np.float32)
    Wo = np.asarray(Wo, dtype=np.float32)
    W1 = np.asarray(W1, dtype=np.float32)
    b1 = np.asarray(b1, dtype=np.float32)
    W2 = np.asarray(W2, dtype=np.float32)
    b2 = np.asarray(b2, dtype=np.float32)
    g_ln1 = np.asarray(g_ln1, dtype=np.float32)
    b_ln1 = np.asarray(b_ln1, dtype=np.float32)
    g_ln2 = np.asarray(g_ln2, dtype=np.float32)
    b_ln2 = np.asarray(b_ln2, dtype=np.float32)
    assert not np.any(b_ln1), "nonzero b_ln1 not supported by this kernel"

    nc = _build()
    mask = _mask_np()
    scale = 1.0 / math.sqrt(DK)

    # LN2 gain folds into W1 (exactly); LN2 bias folds into the fc1 bias.
    W1_eff = g_ln2[:, None] * W1
    b1_eff = b1 + b_ln2 @ W1
    w1_r = np.ascontiguousarray(
        W1_eff.reshape(DT, 128, HT, 128).transpose(1, 2, 0, 3).reshape(128, -1)
    ).astype(NPBF16)
    b1r = np.ascontiguousarray(b1_eff.reshape(HT, 128).T).astype(np.float32)
    w2_bf = W2.astype(NPBF16)
    b2r = b2.reshape(1, D).astype(NPBF16)

    in_maps = []
    for core in range(8):
        b, r = core // G, core % G
        hsl = slice(HG * r, HG * (r + 1))
        # [D, HG, 64] with LN1 gain folded in; Q side also folds 1/sqrt(dk)
        wq3 = (Wq[hsl] * g_ln1[None, :, None]).transpose(1, 0, 2) * scale
        wk3 = (Wk[hsl] * g_ln1[None, :, None]).transpose(1, 0, 2)
        wv3 = (Wv[hsl] * g_ln1[None, :, None]).transpose(1, 0, 2)
        wqk = np.concatenate([wq3, wk3], axis=2)  # [D, HG, 128]
        wqk_r = np.ascontiguousarray(
            wqk.reshape(DT, 128, HG, 128).transpose(1, 0, 2, 3).reshape(128, -1)
        ).astype(NPBF16)
        wv_r = np.ascontiguousarray(
            wv3.reshape(DT, 128, HG, 64).transpose(1, 0, 2, 3).reshape(128, -1)
        ).astype(NPBF16)
        csqk = wqk.sum(axis=0).reshape(1, -1).astype(NPBF16)
        csv = wv3.sum(axis=0).reshape(1, -1).astype(NPBF16)
        wo_c = Wo[HG * DK * r:HG * DK * (r + 1), :]
        wo3 = np.ascontiguousarray(
            wo_c.reshape(HG, DK, D).transpose(1, 0, 2).reshape(DK, HG * D)
        ).astype(NPBF16)
        xb = x[b].astype(NPBF16)
        xT_r = np.ascontiguousarray(
            xb.T.reshape(DT, 128, S).transpose(1, 0, 2).reshape(128, -1)
        )
        # core's MLP rows: 256 from each ReduceScatter chunk
        rows1 = slice(r * 256, (r + 1) * 256)
        rows2 = slice(1024 + r * 256, 1024 + (r + 1) * 256)
        xr = np.concatenate([xb[rows1], xb[rows2]], axis=0)
        in_maps.append({
            "xrows": np.ascontiguousarray(xb),
            "xr": np.ascontiguousarray(xr),
            "xT": xT_r,
            "wqk": wqk_r, "wv": wv_r,
            "csqk": np.ascontiguousarray(csqk), "csv": csv,
            "wo3": wo3,
            "w1": w1_r, "b1r": b1r, "w2": w2_bf, "b2r": b2r,
            "mask": mask,
        })

    trace = bool(int(os.environ.get("BENCH_TRACE", "0")))
    res = run_bass_kernel_spmd(nc, in_maps, core_ids=list(range(8)), trace=trace)
    _cache["last_results"] = res

    out = np.empty((B, S, D), dtype=np.float32)
    for core in range(8):
        b, r = core // G, core % G
        o = res.results[core]["out"]
        out[b, r * 256:(r + 1) * 256, :] = o[0:256]
        out[b, 1024 + r * 256:1024 + (r + 1) * 256, :] = o[256:512]
    return out


# revision 18
# speedup vs baseline: 1.0163x; 1.0140x over previous
"""GPT layer (B=2, S=2048, D=768, H=12, DK=64, HID=3072, causal) on 8 TRN2 cores.

Sharding: cores 0-3 handle batch 0, cores 4-7 batch 1. Within a 4-core group:
tensor-parallel attention over heads (3 heads/core); the W_o partial product is
ReduceScattered in two row-chunks (pipelined against attention / MLP); each
core then runs LN2 + full-width MLP on its own 512 rows (2x 256-row pieces).

All matmuls run in bf16 (fp32 PSUM accumulation). LayerNorm1's mean
subtraction is folded into the QKV projections via an extended contraction
(Q = rstd * (x@Wq - mu*colsum(Wq))), so the transposed activations x^T are
uploaded pre-transposed from the host and never normalized on-device.
Softmax skips max-subtraction (scores are O(1) by construction); its
denominator comes from a ones-column appended to V; the causal mask is a
multiplicative 0/1 bf16 mask applied in-place to the single 128-wide diagonal
block of each diagonal score tile. Score tiles are exp'd in PAIRS
([128,1024] PSUM) to amortize the ~352-cycle ACT instruction overhead.
"""

import math
import os
from contextlib import ExitStack

import numpy as np
import ml_dtypes

import concourse.bass as bass
import concourse.tile as tile
from concourse import bacc, mybir
from concourse.bass_utils import run_bass_kernel_spmd
from concourse.masks import make_identity


F32 = mybir.dt.float32
BF16 = mybir.dt.bfloat16
AF = mybir.ActivationFunctionType
ALU = mybir.AluOpType
NPBF16 = ml_dtypes.bfloat16

B, S, D, H, DK, HID = 2, 2048, 768, 12, 64, 3072
EPS = 1e-5
G = 4            # cores per batch group
HG = H // G      # heads per core (3)
R = S // G       # rows per core (512)
NT = S // 128    # seq tiles (16)
DT = D // 128    # d tiles (6)
HT = HID // 128  # hid tiles (24)
CH = 4           # query chunks of 512

_cache = {}


def _build():
    if "nc" in _cache:
        return _cache["nc"]
    nc = bacc.Bacc("TRN2", target_bir_lowering=False, num_devices=8)

    xrows_d = nc.dram_tensor("xrows", [S, D], BF16, kind="ExternalInput")
    xr_d = nc.dram_tensor("xr", [R, D], BF16, kind="ExternalInput")
    xT_d = nc.dram_tensor("xT", [128, DT * S], BF16, kind="ExternalInput")
    wqk_d = nc.dram_tensor("wqk", [128, DT * HG * 128], BF16, kind="ExternalInput")
    wv_d = nc.dram_tensor("wv", [128, DT * HG * 64], BF16, kind="ExternalInput")
    csqk_d = nc.dram_tensor("csqk", [1, HG * 128], BF16, kind="ExternalInput")
    csv_d = nc.dram_tensor("csv", [1, HG * 64], BF16, kind="ExternalInput")
    wo3_d = nc.dram_tensor("wo3", [64, HG * D], BF16, kind="ExternalInput")
    w1_d = nc.dram_tensor("w1", [128, HT * DT * 128], BF16, kind="ExternalInput")
    b1_d = nc.dram_tensor("b1r", [128, HT], F32, kind="ExternalInput")
    w2_d = nc.dram_tensor("w2", [HID, D], BF16, kind="ExternalInput")
    b2_d = nc.dram_tensor("b2r", [1, D], BF16, kind="ExternalInput")
    mask_d = nc.dram_tensor("mask", [128, 4 * 512], BF16, kind="ExternalInput")
    out_d = nc.dram_tensor("out", [R, D], F32, kind="ExternalOutput")

    with tile.TileContext(nc) as tc, ExitStack() as top:
        consts = top.enter_context(tc.tile_pool(name="consts", bufs=1))
        dram = top.enter_context(tc.tile_pool(name="dram", bufs=1, space="DRAM"))

        ident = consts.tile([128, 128], F32)
        make_identity(nc, ident[:])
        ident_bf = consts.tile([128, 128], BF16)
        make_identity(nc, ident_bf[:])
        ones_bf = consts.tile([1, 128], BF16)
        nc.vector.memset(ones_bf[:], 1.0)
        eps_sb = consts.tile([128, 1], F32)
        nc.vector.memset(eps_sb[:], EPS)
        mask_sb = consts.tile([128, 4, 512], BF16)
        nc.sync.dma_start(mask_sb[:], mask_d[:].rearrange("p (d q) -> p d q", q=512))
        wqk_sb = consts.tile([128, DT, HG, 128], BF16)
        nc.sync.dma_start(
            wqk_sb[:], wqk_d[:].rearrange("p (t h n) -> p t h n", t=DT, h=HG)
        )
        wv_sb = consts.tile([128, DT, HG, 64], BF16)
        nc.sync.dma_start(
            wv_sb[:], wv_d[:].rearrange("p (t h n) -> p t h n", t=DT, h=HG)
        )
        csqk_sb = consts.tile([1, HG, 128], BF16)
        nc.sync.dma_start(csqk_sb[:], csqk_d[:].rearrange("p (h n) -> p h n", h=HG))
        csv_sb = consts.tile([1, HG, 64], BF16)
        nc.sync.dma_start(csv_sb[:], csv_d[:].rearrange("p (h n) -> p h n", h=HG))
        wo3_sb = consts.tile([64, HG, D], BF16)
        nc.sync.dma_start(wo3_sb[:], wo3_d[:].rearrange("p (h n) -> p h n", h=HG))
        b1_sb = consts.tile([128, HT], F32)
        nc.sync.dma_start(b1_sb[:], b1_d[:])
        b2bc = consts.tile([128, D], BF16)
        nc.sync.dma_start(
            b2bc[:],
            bass.AP(tensor=b2_d[:].tensor, offset=b2_d[:].offset, ap=[[0, 128], [1, D]]),
        )

        party1 = dram.tile([2 * R, D], BF16)
        party2 = dram.tile([2 * R, D], BF16)
        rs1 = dram.tile([R // 2, D], BF16)
        rs2 = dram.tile([R // 2, D], BF16)

        with ExitStack() as attn_scope:
            apool = attn_scope.enter_context(tc.tile_pool(name="apool", bufs=1))
            stats = attn_scope.enter_context(tc.tile_pool(name="stats", bufs=8))
            scratch = attn_scope.enter_context(tc.tile_pool(name="scratch", bufs=3))
            epool = attn_scope.enter_context(tc.tile_pool(name="epool", bufs=4))
            ps_sc = attn_scope.enter_context(
                tc.tile_pool(name="ps_sc", bufs=1, space="PSUM")
            )
            ps_o = attn_scope.enter_context(
                tc.tile_pool(name="ps_o", bufs=1, space="PSUM")
            )
            ps_qkv = attn_scope.enter_context(
                tc.tile_pool(name="ps_qkv", bufs=1, space="PSUM")
            )
            ps_w = attn_scope.enter_context(
                tc.tile_pool(name="ps_w", bufs=1, space="PSUM")
            )

            xT_sb = apool.tile([128, DT, S], BF16)
            for dt in range(DT):
                nc.sync.dma_start(
                    xT_sb[:, dt, 0:512], xT_d[:, dt * S: dt * S + 512]
                )

            QT = apool.tile([64, HG, S], BF16)
            KT = apool.tile([64, HG, S], BF16)
            Vg = apool.tile([128, NT, HG, DK + 1], BF16)
            nc.vector.memset(Vg[:, :, :, DK:DK + 1], 1.0)
            OT = apool.tile([64, HG, S], BF16)
            rstd_bc = apool.tile([128, S], F32)
            muT = apool.tile([1, S], BF16)   # -mean, transposed to a row
            rsT = apool.tile([1, S], BF16)   # rstd, transposed to a row
            negmean_all = apool.tile([128, NT], F32)
            rstd_all = apool.tile([128, NT], F32)

            # ---- LN1 statistics from row-layout x (all upfront) ----
            for st in range(NT):
                xt = scratch.tile([128, D], BF16, tag="xin")
                nc.sync.dma_start(xt[:], xrows_d[st * 128:(st + 1) * 128, :])
                bn6 = stats.tile([128, 3, 6], F32, tag="bn6")
                for sg in range(3):
                    nc.vector.bn_stats(bn6[:, sg, :], xt[:, sg * 256:(sg + 1) * 256])
                mv = stats.tile([128, 2], F32, tag="mv")
                nc.vector.bn_aggr(mv[:], bn6[:])
                nc.vector.tensor_scalar_mul(
                    negmean_all[:, st:st + 1], mv[:, 0:1], -1.0
                )
                std = stats.tile([128, 1], F32, tag="std")
                nc.scalar.activation(std[:], mv[:, 1:2], AF.Sqrt, bias=eps_sb[:])
                nc.vector.reciprocal(rstd_all[:, st:st + 1], std[:])

            # remaining x^T chunks (chunk 0 was queued before the stats DMAs)
            for c in range(1, CH):
                for dt in range(DT):
                    nc.sync.dma_start(
                        xT_sb[:, dt, c * 512:(c + 1) * 512],
                        xT_d[:, dt * S + c * 512: dt * S + (c + 1) * 512],
                    )

            # transpose stats to single-partition rows (legal matmul operands),
            # then broadcast rstd along partitions via K=1 matmuls
            for c in range(CH):
                ptm = ps_w.tile([128, 512], F32, tag="pw", bufs=1)
                for j in range(4):
                    nc.tensor.transpose(
                        ptm[0:1, j * 128:(j + 1) * 128],
                        negmean_all[:, 4 * c + j:4 * c + j + 1], ident[:],
                    )
                nc.vector.tensor_copy(muT[:, c * 512:(c + 1) * 512], ptm[0:1, :])
                ptr = ps_w.tile([128, 512], F32, tag="pw", bufs=1)
                for j in range(4):
                    nc.tensor.transpose(
                        ptr[0:1, j * 128:(j + 1) * 128],
                        rstd_all[:, 4 * c + j:4 * c + j + 1], ident[:],
                    )
                nc.vector.tensor_copy(rsT[:, c * 512:(c + 1) * 512], ptr[0:1, :])
                pbx = ps_w.tile([128, 512], F32, tag="pw", bufs=1)
                for j in range(4):
                    nc.tensor.matmul(
                        pbx[:, j * 128:(j + 1) * 128], ones_bf[:],
                        rsT[:, (4 * c + j) * 128:(4 * c + j + 1) * 128],
                        start=True, stop=True, skip_group_check=True,
                    )
                nc.vector.tensor_copy(rstd_bc[:, c * 512:(c + 1) * 512], pbx[:])

            # ---- attention, chunk-pipelined over query blocks of 512 ----
            for c in range(CH):
                cs = c * 512
                # QK projections for this chunk (packed Q|K per head)
                for h in range(HG):
                    pqk = ps_qkv.tile([128, 512], F32, tag="pqk", bufs=1)
                    for dt in range(DT):
                        nc.tensor.matmul(
                            pqk[:], wqk_sb[:, dt, h, :], xT_sb[:, dt, cs:cs + 512],
                            start=(dt == 0), stop=False,
                        )
                    for j in range(4):
                        nc.tensor.matmul(
                            pqk[:, j * 128:(j + 1) * 128], csqk_sb[:, h, :],
                            muT[:, (4 * c + j) * 128:(4 * c + j + 1) * 128],
                            start=False, stop=True, skip_group_check=True,
                        )
                    nc.vector.tensor_mul(
                        QT[:, h, cs:cs + 512], pqk[0:64, :],
                        rstd_bc[0:64, cs:cs + 512],
                    )
                    nc.vector.tensor_mul(
                        KT[:, h, cs:cs + 512], pqk[64:128, :],
                        rstd_bc[64:128, cs:cs + 512],
                    )
                # V for the 4 key tiles of this chunk
                for j in range(4):
                    st = 4 * c + j
                    pv = ps_qkv.tile([128, HG * 64], F32, tag="pv", bufs=1)
                    for dt in range(DT):
                        nc.tensor.matmul(
                            pv[:], xT_sb[:, dt, st * 128:(st + 1) * 128],
                            wv_sb[:, dt, :, :],
                            start=(dt == 0), stop=False,
                        )
                    nc.tensor.matmul(
                        pv[:], muT[:, st * 128:(st + 1) * 128], csv_sb[:, :, :],
                        start=False, stop=True,
                    )
                    nc.vector.tensor_scalar(
                        Vg[:, st, :, 0:DK],
                        pv[:].rearrange("p (h n) -> p h n", h=HG),
                        rstd_all[:, st:st + 1], None, op0=ALU.mult,
                    )

                # scores -> exp -> (mask) -> A@V, software-pipelined
                ntl = 4 * (c + 1)
                LEAD = 2
                for h in range(HG):
                    po = ps_o.tile([DK + 1, 512], F32, tag="po", bufs=2)
                    es = {}
                    q0s = {}
                    for i in range(ntl + LEAD):
                        if i < ntl:
                            t = i
                            dd = t - 4 * c
                            # diagonal-band tiles: queries < dd*128 are fully
                            # masked, so compute only the live query range
                            q0 = dd * 128 if dd > 0 else 0
                            q0s[t] = q0
                            psc = ps_sc.tile([128, 512], F32, tag="psc", bufs=3)
                            nc.tensor.matmul(
                                psc[:, q0:512], KT[:, h, t * 128:(t + 1) * 128],
                                QT[:, h, cs + q0:cs + 512], start=True, stop=True,
                            )
                            if dd >= 0:
                                e_r = epool.tile([128, 512], BF16, tag="e", bufs=4)
                                nc.scalar.activation(
                                    e_r[:, q0:512], psc[:, q0:512], AF.Exp
                                )
                                e_t = epool.tile([128, 512], BF16, tag="em", bufs=4)
                                nc.vector.tensor_mul(
                                    e_t[:, q0:512], e_r[:, q0:512],
                                    mask_sb[:, dd, q0:512],
                                )
                            else:
                                e_t = epool.tile([128, 512], BF16, tag="e", bufs=4)
                                nc.scalar.activation(e_t[:], psc[:], AF.Exp)
                            es[t] = e_t
                        j = i - LEAD
                        if j >= 0:
                            jq0 = q0s.pop(j)
                            nc.tensor.matmul(
                                po[:, jq0:512], Vg[:, j, h, :],
                                es.pop(j)[:, jq0:512],
                                start=(j == 0), stop=(j == ntl - 1),
                            )
                    # normalize: OT = po[:DK] * broadcast(1/po[DK])
                    den_s = stats.tile([1, 512], F32, tag="den")
                    nc.vector.tensor_copy(den_s[:], po[DK:DK + 1, :])
                    rec_f = stats.tile([1, 512], F32, tag="rec_f")
                    nc.vector.reciprocal_approx_fast(rec_f[:], den_s[:])
                    rec = stats.tile([1, 512], BF16, tag="rec")
                    nc.vector.tensor_copy(rec[:], rec_f[:])
                    rb = epool.tile([64, 512], BF16, tag="rb", bufs=2)
                    nc.gpsimd.partition_broadcast(rb[:], rec[:])
                    nc.vector.tensor_mul(OT[:, h, cs:cs + 512], po[0:DK, :], rb[:])

                # W_o partial for this chunk's 4 row tiles
                for rt in range(4 * c, 4 * c + 4):
                    party_d = party1 if rt < 8 else party2
                    row0 = (rt % 8) * 128
                    for n0, nw in ((0, 512), (512, 256)):
                        pw = ps_w.tile([128, 512], F32, tag="pw", bufs=1)
                        for hh in range(HG):
                            nc.tensor.matmul(
                                pw[:, 0:nw], OT[:, hh, rt * 128:(rt + 1) * 128],
                                wo3_sb[:, hh, n0:n0 + nw],
                                start=(hh == 0), stop=(hh == HG - 1),
                            )
                        prow = scratch.tile([128, 512], BF16, tag="prow")
                        nc.vector.tensor_copy(prow[:, 0:nw], pw[:, 0:nw])
                        nc.sync.dma_start(
                            party_d[row0:row0 + 128, n0:n0 + nw], prow[:, 0:nw]
                        )

                if c == 1:
                    nc.gpsimd.collective_compute(
                        "ReduceScatter", ALU.add,
                        replica_groups=[[0, 1, 2, 3], [4, 5, 6, 7]],
                        ins=[party1[:].opt()], outs=[rs1[:].opt()],
                    )

        nc.gpsimd.collective_compute(
            "ReduceScatter", ALU.add,
            replica_groups=[[0, 1, 2, 3], [4, 5, 6, 7]],
            ins=[party2[:].opt()], outs=[rs2[:].opt()],
        )

        # ---- LN2 + MLP over two 256-row halves ----
        with ExitStack() as mlp_scope:
            mpool = mlp_scope.enter_context(tc.tile_pool(name="mpool", bufs=1))
            mstats = mlp_scope.enter_context(tc.tile_pool(name="mstats", bufs=8))
            mscratch = mlp_scope.enter_context(tc.tile_pool(name="mscratch", bufs=3))
            w1pool = mlp_scope.enter_context(tc.tile_pool(name="w1pool", bufs=3))
            w2pool = mlp_scope.enter_context(tc.tile_pool(name="w2pool", bufs=3))
            ps_t2 = mlp_scope.enter_context(
                tc.tile_pool(name="ps_t2", bufs=1, space="PSUM")
            )
            ps_f1 = mlp_scope.enter_context(
                tc.tile_pool(name="ps_f1", bufs=1, space="PSUM")
            )
            ps_f2 = mlp_scope.enter_context(
                tc.tile_pool(name="ps_f2", bufs=1, space="PSUM")
            )

            y_sb = mpool.tile([128, 4, D], BF16)
            h2T = mpool.tile([128, DT, R], BF16)
            gT = mpool.tile([128, HT, R], BF16)

            for hf in range(2):
                rs_d = rs1 if hf == 0 else rs2
                for m in range(2):
                    rl = hf * 256 + m * 128  # local row offset
                    rs_t = mscratch.tile([128, D], BF16, tag="rst")
                    nc.sync.dma_start(rs_t[:], rs_d[m * 128:(m + 1) * 128, :])
                    xr_t = mscratch.tile([128, D], BF16, tag="xrt")
                    nc.sync.dma_start(xr_t[:], xr_d[rl:rl + 128, :])
                    nc.vector.tensor_add(y_sb[:, hf * 2 + m, :], rs_t[:], xr_t[:])
                    bn6 = mstats.tile([128, 3, 6], F32, tag="bn6")
                    for sg in range(3):
                        nc.vector.bn_stats(
                            bn6[:, sg, :],
                            y_sb[:, hf * 2 + m, sg * 256:(sg + 1) * 256],
                        )
                    mv = mstats.tile([128, 2], F32, tag="mv")
                    nc.vector.bn_aggr(mv[:], bn6[:])
                    std = mstats.tile([128, 1], F32, tag="std")
                    nc.scalar.activation(std[:], mv[:, 1:2], AF.Sqrt, bias=eps_sb[:])
                    rstd = mstats.tile([128, 1], F32, tag="rstd")
                    nc.vector.reciprocal(rstd[:], std[:])
                    h2_t = mscratch.tile([128, D], BF16, tag="h2row")
                    nc.vector.tensor_scalar(
                        h2_t[:], y_sb[:, hf * 2 + m, :], mv[:, 0:1], rstd[:],
                        op0=ALU.subtract, op1=ALU.mult,
                    )
                    for half in range(2):
                        pt = ps_t2.tile([128, 384], BF16, tag="ptr", bufs=2)
                        for k in range(3):
                            dt = half * 3 + k
                            nc.tensor.transpose(
                                pt[:, k * 128:(k + 1) * 128],
                                h2_t[:, dt * 128:(dt + 1) * 128], ident_bf[:],
                            )
                        nc.vector.tensor_copy(
                            h2T[:, half * 3:half * 3 + 3, rl:rl + 128], pt[:]
                        )

                # fc1 + gelu for this half
                for hc in range(HT):
                    w1c = w1pool.tile([128, DT, 128], BF16, tag="w1c")
                    nc.sync.dma_start(
                        w1c[:],
                        w1_d[:, hc * DT * 128:(hc + 1) * DT * 128]
                        .rearrange("p (t n) -> p t n", t=DT),
                    )
                    pf = ps_f1.tile([128, 256], F32, tag="pf", bufs=2)
                    for dt in range(DT):
                        nc.tensor.matmul(
                            pf[:], w1c[:, dt, :],
                            h2T[:, dt, hf * 256:(hf + 1) * 256],
                            start=(dt == 0), stop=(dt == DT - 1),
                        )
                    nc.scalar.activation(
                        gT[:, hc, hf * 256:(hf + 1) * 256], pf[:], AF.Gelu,
                        bias=b1_sb[:, hc:hc + 1],
                    )

                # fc2 for this half (W2 streamed)
                pacc = {}
                for m in range(2):
                    for n0, nw in ((0, 512), (512, 256)):
                        pacc[(m, n0)] = ps_f2.tile(
                            [128, nw], F32, tag=f"pf2_{m}_{n0}", bufs=1,
                            name=f"pf2_{m}_{n0}",
                        )
                for t in range(HT):
                    w2t = w2pool.tile([128, D], BF16, tag="w2t")
                    nc.sync.dma_start(w2t[:], w2_d[t * 128:(t + 1) * 128, :])
                    for m in range(2):
                        rl = hf * 256 + m * 128
                        for n0, nw in ((0, 512), (512, 256)):
                            nc.tensor.matmul(
                                pacc[(m, n0)], gT[:, t, rl:rl + 128],
                                w2t[:, n0:n0 + nw],
                                start=(t == 0), stop=(t == HT - 1),
                            )
                for m in range(2):
                    yb = mscratch.tile([128, D], BF16, tag="yb")
                    nc.vector.tensor_add(yb[:], y_sb[:, hf * 2 + m, :], b2bc[:])
                    o_t = mscratch.tile([128, D], F32, tag="ot")
                    for n0, nw in ((0, 512), (512, 256)):
                        nc.vector.tensor_add(
                            o_t[:, n0:n0 + nw], pacc[(m, n0)], yb[:, n0:n0 + nw]
                        )
                    nc.sync.dma_start(
                        out_d[(hf * 2 + m) * 128:(hf * 2 + m + 1) * 128, :], o_t[:]
                    )

    nc.finalize()
    _cache["nc"] = nc
    return nc


def _mask_np():
    # mask[p, dd*512 + q] = 1 where key (dd*128 + p) <= query q, else 0
    m = np.zeros((128, 4 * 512), dtype=NPBF16)
    p = np.arange(128)[:, None]
    q = np.arange(512)[None, :]
    for dd in range(4):
        m[:, dd * 512:(dd + 1) * 512] = (dd * 128 + p <= q).astype(NPBF16)
    return m


def kernel(x, Wq, Wk, Wv, Wo, W1, b1, W2, b2, g_ln1, b_ln1, g_ln2, b_ln2):
    x = np.asarray(x, dtype=np.float32)
    Wq = np.asarray(Wq, dtype=np.float32)
    Wk = np.asarray(Wk, dtype=np.float32)
    Wv = np.asarray(Wv, dtype=np.float32)
    Wo = np.asarray(Wo, dtype=np.float32)
    W1 = np.asarray(W1, dtype=np.float32)
    b1 = np.asarray(b1, dtype=np.float32)
    W2 = np.asarray(W2, dtype=np.float32)
    b2 = np.asarray(b2, dtype=np.float32)
    g_ln1 = np.asarray(g_ln1, dtype=np.float32)
    b_ln1 = np.asarray(b_ln1, dtype=np.float32)
    g_ln2 = np.asarray(g_ln2, dtype=np.float32)
    b_ln2 = np.asarray(b_ln2, dtype=np.float32)
    assert not np.any(b_ln1), "nonzero b_ln1 not supported by this kernel"

    nc = _build()
    mask = _mask_np()
    scale = 1.0 / math.sqrt(DK)

    # LN2 gain folds into W1 (exactly); LN2 bias folds into the fc1 bias.
    W1_eff = g_ln2[:, None] * W1
    b1_eff = b1 + b_ln2 @ W1
    w1_r = np.ascontiguousarray(
        W1_eff.reshape(DT, 128, HT, 128).transpose(1, 2, 0, 3).reshape(128, -1)
    ).astype(NPBF16)
    b1r = np.ascontiguousarray(b1_eff.reshape(HT, 128).T).astype(np.float32)
    w2_bf = W2.astype(NPBF16)
    b2r = b2.reshape(1, D).astype(NPBF16)

    in_maps = []
    for core in range(8):
        b, r = core // G, core % G
        hsl = slice(HG * r, HG * (r + 1))
        # [D, HG, 64] with LN1 gain folded in; Q side also folds 1/sqrt(dk)
        wq3 = (Wq[hsl] * g_ln1[None, :, None]).transpose(1, 0, 2) * scale
        wk3 = (Wk[hsl] * g_ln1[None, :, None]).transpose(1, 0, 2)
        wv3 = (Wv[hsl] * g_ln1[None, :, None]).transpose(1, 0, 2)
        wqk = np.concatenate([wq3, wk3], axis=2)  # [D, HG, 128]
        wqk_r = np.ascontiguousarray(
            wqk.reshape(DT, 128, HG, 128).transpose(1, 0, 2, 3).reshape(128, -1)
        ).astype(NPBF16)
        wv_r = np.ascontiguousarray(
            wv3.reshape(DT, 128, HG, 64).transpose(1, 0, 2, 3).reshape(128, -1)
        ).astype(NPBF16)
        csqk = wqk.sum(axis=0).reshape(1, -1).astype(NPBF16)
        csv = wv3.sum(axis=0).reshape(1, -1).astype(NPBF16)
        wo_c = Wo[HG * DK * r:HG * DK * (r + 1), :]
        wo3 = np.ascontiguousarray(
            wo_c.reshape(HG, DK, D).transpose(1, 0, 2).reshape(DK, HG * D)
        ).astype(NPBF16)
        xb = x[b].astype(NPBF16)
        xT_r = np.ascontiguousarray(
            xb.T.reshape(DT, 128, S).transpose(1, 0, 2).reshape(128, -1)
        )
        # core's MLP rows: 256 from each ReduceScatter chunk
        rows1 = slice(r * 256, (r + 1) * 256)
        rows2 = slice(1024 + r * 256, 1024 + (r + 1) * 256)
        xr = np.concatenate([xb[rows1], xb[rows2]], axis=0)
        in_maps.append({
            "xrows": np.ascontiguousarray(xb),
            "xr": np.ascontiguousarray(xr),
            "xT": xT_r,
            "wqk": wqk_r, "wv": wv_r,
            "csqk": np.ascontiguousarray(csqk), "csv": csv,
            "wo3": wo3,
            "w1": w1_r, "b1r": b1r, "w2": w2_bf, "b2r": b2r,
            "mask": mask,
        })

    trace = bool(int(os.environ.get("BENCH_TRACE", "0")))
    res = run_bass_kernel_spmd(nc, in_maps, core_ids=list(range(8)), trace=trace)
    _cache["last_results"] = res

    out = np.empty((B, S, D), dtype=np.float32)
    for core in range(8):
        b, r = core // G, core % G
        o = res.results[core]["out"]
        out[b, r * 256:(r + 1) * 256, :] = o[0:256]
        out[b, 1024 + r * 256:1024 + (r + 1) * 256, :] = o[256:512]
    return out
